# revision 24
# baseline (speedup 1.0000x reference)
"""Trainium2 Bass kernel for nn_Encoder_Block (graph-transformer encoder block).

Sharding: data-parallel over batch b — core c handles batch c (B=8, 8 cores).

Per-core dataflow (b fixed; N=128 nodes, C=256, H*DK=C, HID=1024):
  phase 0 : x1 = LN1(x); qT/kT/vT = (x1 @ Wqkv + b)^T via PE (x1 transposed
            through a DRAM round-trip + DMA-xbar transpose).
  main loop over i-blocks of G=4 rows of the edge tensor:
    eT   = We^T-form matmul on host-pre-transposed y (f32r, full fp32 bits)
    u    = Square(e + be + 0.5)  (ACT, per-partition bias)     [bf16]
    uq   = (u - 0.25) * q_i      (DVE fused tensor_scalar)
    attn = uq * kT               (DVE tensor_tensor, bcast AP over g)
    s    = Exp(attn)             (ACT)
    sumexp / wsum = fused tensor_scalar / scalar_tensor_tensor accum_out
    agg column = wsum * recip(sumexp)
    edge_out natural [j,c] via attn-as-lhsT matmuls + boe ones-row matmul
            + y residual via identity matmul (f32r) accumulated in PSUM
    ln4  = bn_stats(PSUM) + ln/exp rstd + fused tensor_scalar apply -> y2
    y2T  via DRAM round-trip + xbar transpose (contiguous [128,512] dests)
    mlp  = relu(W1^T-form) (ACT bias) -> h^T ; out natural via h-as-lhsT
            + b2 ones-row + y2 identity-residual in PSUM
    ln6  -> y_out -> DMA out
  tail    : node_out from agg, ln3, mlp_x, ln5 -> x_out.

LN scale/bias: setup_inputs uses s=1, b=0 for all LNs. ln1/ln3 are applied
generally on device (one-time broadcast tiles). ln4 gets a device fallback
path if nontrivial; ln5/ln6 are host-fixable (final ops).
"""
import sys
import numpy as np

sys.path.insert(0, "/opt/trn_rl_repo")

import ml_dtypes  # noqa: E402

import concourse.bacc as bacc  # noqa: E402
import concourse.bass as bass  # noqa: E402
import concourse.tile as tile  # noqa: E402
from concourse import mybir  # noqa: E402
from concourse.bass_utils import run_bass_kernel_spmd  # noqa: E402

# Steer every ACT function this kernel uses into one table set
# (natural_log_exp_and_others) so the whole kernel runs with a single
# ACT_TABLE_LOAD instead of ping-ponging between sets on every Ln/Exp.
# Only the python-side chooser dict is edited; set ids (insertion order)
# stay aligned with the act_info.json walrus reads, so the loads remain
# valid — the chooser just loses the option of picking a set that lacks
# one of our functions.
import concourse.hw_specs as _hw_specs  # noqa: E402

_ORIG_GAT = _hw_specs.get_activation_tables
_MY_SET = "natural_log_exp_and_others"


def _patched_gat(arch):
    tables = _ORIG_GAT(arch)
    mine = tables.get(_MY_SET)
    if mine:
        my_funcs = {
            mybir.ActivationFunctionType.Square,
            mybir.ActivationFunctionType.Exp,
            mybir.ActivationFunctionType.Ln,
            mybir.ActivationFunctionType.Relu,
            mybir.ActivationFunctionType.Identity,
        } & mine
        tables = {
            name: (funcs if name == _MY_SET else funcs - my_funcs)
            for name, funcs in tables.items()
        }
    return tables


bacc.get_activation_tables = _patched_gat

FP32 = mybir.dt.float32
F32R = mybir.dt.float32r
BF16 = mybir.dt.bfloat16
AF = mybir.ActivationFunctionType
OP = mybir.AluOpType
BF16NP = ml_dtypes.bfloat16

B, N, C, HID = 8, 128, 256, 1024
DK = 32
G = 4                      # i-rows per block
NBLK = N // G              # 32
CH = C // 128              # 2 chunks of the feature dim
MH = HID // 128            # 8 chunks of the mlp hidden dim
EPS = 1e-5

_BUILD_CACHE = {}


def _bcast_g(ap_2d, g):
    """[p, f] AP -> [p, g, f] with stride-0 middle dim."""
    return bass.AP(tensor=ap_2d.tensor, offset=ap_2d.offset,
                   ap=[ap_2d.ap[0], [0, g], ap_2d.ap[1]])


def _build(apply_ln4sb: bool):
    nc = bacc.Bacc()

    dp = nc.declare_dram_parameter
    x_d = dp("x", [N, C], FP32, isOutput=False)
    ynat_d = dp("ynat", [N, N, C], F32R, isOutput=False)
    ytr_d = dp("ytr", [NBLK, CH, 128, G, 128], F32R, isOutput=False)
    we_d = dp("we", [CH, 128, C], F32R, isOutput=False)
    woe_d = dp("woe", [CH, 128, C], BF16, isOutput=False)
    wq_d = dp("wq", [CH, 128, C], BF16, isOutput=False)
    wk_d = dp("wk", [CH, 128, C], BF16, isOutput=False)
    wv_d = dp("wv", [CH, 128, C], BF16, isOutput=False)
    won_d = dp("won", [CH, 128, C], F32R, isOutput=False)
    w1y_d = dp("w1y", [CH, 128, HID], BF16, isOutput=False)
    w2y_d = dp("w2y", [MH, 128, C], BF16, isOutput=False)
    w1x_d = dp("w1x", [CH, 128, HID], BF16, isOutput=False)
    w2x_d = dp("w2x", [MH, 128, C], BF16, isOutput=False)
    bqs_d = dp("bqs", [128, CH], FP32, isOutput=False)
    bks_d = dp("bks", [128, CH], FP32, isOutput=False)
    bvs_d = dp("bvs", [128, CH], FP32, isOutput=False)
    be5_d = dp("be5", [128, CH], FP32, isOutput=False)
    b1y_d = dp("b1y", [128, MH], FP32, isOutput=False)
    b1x_d = dp("b1x", [128, MH], FP32, isOutput=False)
    boe_d = dp("boe", [1, C], F32R, isOutput=False)
    b2y_d = dp("b2y", [1, C], F32R, isOutput=False)
    bon_d = dp("bon", [1, C], F32R, isOutput=False)
    b2x_d = dp("b2x", [1, C], F32R, isOutput=False)
    ones_d = dp("ones1", [1, 128], F32R, isOutput=False)
    ident_d = dp("ident", [128, 128], F32R, isOutput=False)
    identb_d = dp("identb", [128, 128], BF16, isOutput=False)
    ln1s_d = dp("ln1s", [1, C], FP32, isOutput=False)
    ln1b_d = dp("ln1b", [1, C], FP32, isOutput=False)
    ln3s_d = dp("ln3s", [1, C], FP32, isOutput=False)
    ln3b_d = dp("ln3b", [1, C], FP32, isOutput=False)
    ln4s_d = dp("ln4s", [1, C], FP32, isOutput=False)
    ln4b_d = dp("ln4b", [1, C], FP32, isOutput=False)

    xo_d = dp("xo", [N, C], FP32, isOutput=True)
    yo_d = dp("yo", [N, N, C], FP32, isOutput=True)
    import os
    _dbg = bool(os.environ.get("KDEBUG"))
    if _dbg:
        dbg_y2_d = dp("dbg_y2", [N, N, C], FP32, isOutput=True)
        dbg_r5_d = dp("dbg_r5", [N, N, C], FP32, isOutput=True)
        dbg_yt_d = dp("dbg_yt", [NBLK, CH, 128, G * 128], FP32, isOutput=True)
        dbg_h_d = dp("dbg_h", [NBLK, MH, 128, G * 128], FP32, isOutput=True)

    with tile.TileContext(nc) as tc:
        with (
            tc.tile_pool(name="persist", bufs=1) as pp,
            tc.tile_pool(name="pdram", bufs=1) as pd,
        ):
            # ---------- persistent loads ----------
            def pload(name, dram, shape, dt):
                t = pp.tile(shape, dt, name=name, tag=name)
                nc.sync.dma_start(out=t, in_=dram[:].rearrange("k p c -> p k c"))
                return t

            wq_sb = pload("wq_sb", wq_d, [128, CH, C], BF16)
            wk_sb = pload("wk_sb", wk_d, [128, CH, C], BF16)
            wv_sb = pload("wv_sb", wv_d, [128, CH, C], BF16)
            we_sb = pload("we_sb", we_d, [128, CH, C], F32R)
            woe_sb = pload("woe_sb", woe_d, [128, CH, C], BF16)
            w1y_sb = pload("w1y_sb", w1y_d, [128, CH, HID], BF16)
            w2y_sb = pload("w2y_sb", w2y_d, [128, MH, C], BF16)
            won_sb = pload("won_sb", won_d, [128, CH, C], F32R)
            w1x_sb = pload("w1x_sb", w1x_d, [128, CH, HID], BF16)
            w2x_sb = pload("w2x_sb", w2x_d, [128, MH, C], BF16)

            def sload(name, dram, shape, dt):
                t = pp.tile(shape, dt, name=name, tag=name)
                nc.sync.dma_start(out=t, in_=dram[:])
                return t

            bqs = sload("bqs_sb", bqs_d, [128, CH], FP32)
            bks = sload("bks_sb", bks_d, [128, CH], FP32)
            bvs = sload("bvs_sb", bvs_d, [128, CH], FP32)
            be5 = sload("be5_sb", be5_d, [128, CH], FP32)
            b1y = sload("b1y_sb", b1y_d, [128, MH], FP32)
            b1x = sload("b1x_sb", b1x_d, [128, MH], FP32)
            boe = sload("boe_sb", boe_d, [1, C], F32R)
            b2y = sload("b2y_sb", b2y_d, [1, C], F32R)
            bon = sload("bon_sb", bon_d, [1, C], F32R)
            b2x = sload("b2x_sb", b2x_d, [1, C], F32R)
            ones1 = sload("ones1_sb", ones_d, [1, 128], F32R)
            ident = sload("ident_sb", ident_d, [128, 128], F32R)
            identb = sload("identb_sb", identb_d, [128, 128], BF16)

            def bcload(name, dram):
                t = pp.tile([128, C], FP32, name=name, tag=name)
                src = dram[:]
                bc = bass.AP(tensor=src.tensor, offset=src.offset,
                             ap=[[0, 128], src.ap[1]])
                nc.gpsimd.dma_start(out=t, in_=bc)
                return t

            ln1s = bcload("ln1s_sb", ln1s_d)
            ln1b = bcload("ln1b_sb", ln1b_d)
            ln3s = bcload("ln3s_sb", ln3s_d)
            ln3b = bcload("ln3b_sb", ln3b_d)
            if apply_ln4sb:
                ln4s = bcload("ln4s_sb", ln4s_d)
                ln4b = bcload("ln4b_sb", ln4b_d)

            epsb = pp.tile([128, 1], FP32, name="epsb", tag="epsb")
            nc.vector.memset(epsb, EPS)

            # persistent x-stream tiles
            x1 = pp.tile([128, C], F32R, name="x1t", tag="x1t")
            qs = pp.tile([128, CH, 128], FP32, name="qs", tag="qs")
            kT = pp.tile([128, CH, 128], BF16, name="kT", tag="kT")
            vT = pp.tile([128, CH, 128], BF16, name="vT", tag="vT")
            agg = pp.tile([128, CH, 128], F32R, name="agg", tag="agg")
            x1T0 = pp.tile([128, 128], BF16, name="x1T0", tag="x1T0")
            x1T1 = pp.tile([128, 128], BF16, name="x1T1", tag="x1T1")
            x1Ts = [x1T0, x1T1]

            def layer_norm(dst, src, tagp, scale_t=None, bias_t=None, n_g=1,
                           g_len=C):
                """LN over last axis for [128, n_g, g_len] src (may be PSUM).
                dst fp32 SBUF same shape. Returns nothing."""
                mv = pp.tile([128, n_g, 2], FP32, name=f"{tagp}_mv", tag=f"{tagp}_mv")
                for g in range(n_g):
                    st = pp.tile([128, 6], FP32, name=f"{tagp}_st{g}", tag=f"{tagp}_st", bufs=2)
                    src_g = src[:, g, :] if n_g > 1 else src
                    nc.vector.bn_stats(out=st, in_=src_g)
                    nc.vector.bn_aggr(out=mv[:, g, :], in_=st)
                lnv = pp.tile([128, n_g], FP32, name=f"{tagp}_lnv", tag=f"{tagp}_lnv")
                nc.scalar.activation(out=lnv, in_=mv[:, :, 1], func=AF.Ln,
                                     bias=epsb[:], scale=1.0)
                rstd = pp.tile([128, n_g], FP32, name=f"{tagp}_rstd", tag=f"{tagp}_rstd")
                nc.scalar.activation(out=rstd, in_=lnv, func=AF.Exp,
                                     bias=0.0, scale=-0.5)
                for g in range(n_g):
                    src_g = src[:, g, :] if n_g > 1 else src
                    dst_g = dst[:, g, :] if n_g > 1 else dst
                    nc.vector.tensor_scalar(out=dst_g, in0=src_g,
                                            scalar1=mv[:, g, 0:1],
                                            scalar2=rstd[:, g:g + 1],
                                            op0=OP.subtract, op1=OP.mult)
                if scale_t is not None:
                    for g in range(n_g):
                        dst_g = dst[:, g, :] if n_g > 1 else dst
                        nc.vector.tensor_tensor(out=dst_g, in0=dst_g, in1=scale_t, op=OP.mult)
                        nc.vector.tensor_tensor(out=dst_g, in0=dst_g, in1=bias_t, op=OP.add)

            # ---------- phase 0: x1, qT/kT/vT ----------
            with (
                tc.tile_pool(name="p0", bufs=1) as p0,
                tc.tile_pool(name="p0ps", bufs=1, space="PSUM") as p0ps,
            ):
                xt = p0.tile([128, C], FP32, name="xt")
                nc.sync.dma_start(out=xt, in_=x_d[:])
                layer_norm(x1, xt, "ln1", scale_t=ln1s, bias_t=ln1b)

                x1bf = p0.tile([128, C], BF16, name="x1bf")
                nc.vector.tensor_copy(out=x1bf, in_=x1)
                x1tp = p0ps.tile([128, CH, 128], BF16, name="x1tp")
                for ch in range(CH):
                    nc.tensor.transpose(x1tp[:, ch, :],
                                        x1bf[:, ch * 128:(ch + 1) * 128], identb[:])
                    nc.vector.tensor_copy(out=x1Ts[ch], in_=x1tp[:, ch, :])

                for (w_sb, b_sb, outt, odt) in ((wq_sb, bqs, qs, FP32),
                                                (wk_sb, bks, kT, BF16),
                                                (wv_sb, bvs, vT, BF16)):
                    for mch in range(CH):
                        qps = p0ps.tile([128, 128], FP32, name="qps", tag="qps")
                        for kch in range(CH):
                            nc.tensor.matmul(qps[:], lhsT=w_sb[:, kch, mch * 128:(mch + 1) * 128],
                                             rhs=x1Ts[kch][:],
                                             start=(kch == 0), stop=(kch == CH - 1))
                        nc.scalar.activation(out=outt[:, mch, :], in_=qps[:],
                                             func=AF.Identity, bias=b_sb[:, mch:mch + 1],
                                             scale=1.0)

            # ---------- main loop over edge-row blocks ----------
            with (
                tc.tile_pool(name="lp", bufs=(2 if _dbg else 3)) as lp,
                tc.tile_pool(name="lp3", bufs=(2 if _dbg else 3)) as lp3,
                tc.tile_pool(name="eps_pool", bufs=2, space="PSUM") as eps_pool,
                tc.tile_pool(name="edge_pool", bufs=1, space="PSUM") as edge_pool,
                tc.tile_pool(name="h_pool", bufs=2, space="PSUM") as h_pool,
                tc.tile_pool(name="r5_pool", bufs=1, space="PSUM") as r5_pool,
            ):
                for blk in range(NBLK):
                    i0 = blk * G
                    # loads
                    ytr_t = lp.tile([128, CH, G, 128], F32R, name="ytr_t", tag="ytr_t")
                    nc.sync.dma_start(out=ytr_t, in_=ytr_d[blk].rearrange("k p g j -> p k g j"))
                    ynat_t = lp.tile([128, G, C], F32R, name="ynat_t", tag="ynat_t")
                    nc.sync.dma_start(out=ynat_t, in_=ynat_d[i0:i0 + G].rearrange("g j c -> j g c"))

                    # e projection (transposed, f32r)
                    e_ps = []
                    for mch in range(CH):
                        ep = eps_pool.tile([128, G * 128], FP32, name=f"e_ps{mch}", tag="e_ps")
                        for kch in range(CH):
                            nc.tensor.matmul(ep[:], lhsT=we_sb[:, kch, mch * 128:(mch + 1) * 128],
                                             rhs=ytr_t[:, kch, :, :],
                                             start=(kch == 0), stop=(kch == CH - 1))
                        e_ps.append(ep)

                    # u = (e + be + 0.5)^2  [bf16]
                    u_bf = lp.tile([128, CH, G * 128], BF16, name="u_bf", tag="u_bf")
                    for mch in range(CH):
                        nc.scalar.activation(out=u_bf[:, mch, :], in_=e_ps[mch][:],
                                             func=AF.Square, bias=be5[:, mch:mch + 1],
                                             scale=1.0)

                    # uq = (u - 0.25) * q_i ; attn = uq * kT
                    uq_bf = lp.tile([128, CH, G, 128], BF16, name="uq_bf", tag="uq_bf")
                    for mch in range(CH):
                        for g in range(G):
                            nc.vector.tensor_scalar(
                                out=uq_bf[:, mch, g, :],
                                in0=u_bf[:, mch, g * 128:(g + 1) * 128],
                                scalar1=0.25, scalar2=qs[:, mch, i0 + g:i0 + g + 1],
                                op0=OP.subtract, op1=OP.mult)
                    attn_bf = lp.tile([128, CH, G, 128], BF16, name="attn_bf", tag="attn_bf")
                    for mch in range(CH):
                        nc.vector.tensor_tensor(out=attn_bf[:, mch, :, :],
                                                in0=uq_bf[:, mch, :, :],
                                                in1=_bcast_g(kT[:, mch, :], G), op=OP.mult)

                    # softmax pieces
                    s_bf = lp.tile([128, CH, G, 128], BF16, name="s_bf", tag="s_bf")
                    nc.scalar.activation(out=s_bf.rearrange("p m g j -> p (m g j)"),
                                         in_=attn_bf.rearrange("p m g j -> p (m g j)"),
                                         func=AF.Exp, bias=0.0, scale=1.0)
                    se = lp.tile([128, CH, G], FP32, name="se", tag="se")
                    ws = lp.tile([128, CH, G], FP32, name="ws", tag="ws")
                    scr = lp.tile([128, 128], BF16, name="scr", tag="scr")
                    scr2 = lp.tile([128, 128], BF16, name="scr2", tag="scr2")
                    for mch in range(CH):
                        for g in range(G):
                            nc.vector.tensor_scalar(out=scr, in0=s_bf[:, mch, g, :],
                                                    scalar1=1.0, scalar2=0.0,
                                                    op0=OP.mult, op1=OP.add,
                                                    accum_out=se[:, mch, g:g + 1])
                            nc.vector.scalar_tensor_tensor(out=scr2, in0=s_bf[:, mch, g, :],
                                                           scalar=1.0, in1=vT[:, mch, :],
                                                           op0=OP.mult, op1=OP.mult,
                                                           accum_out=ws[:, mch, g:g + 1])
                    rse = lp.tile([128, CH, G], FP32, name="rse", tag="rse")
                    nc.vector.reciprocal(out=rse, in_=se)
                    for mch in range(CH):
                        nc.vector.tensor_tensor(out=agg[:, mch, i0:i0 + G],
                                                in0=ws[:, mch, :], in1=rse[:, mch, :],
                                                op=OP.mult)

                    # edge_out natural + boe + y residual, accumulate in PSUM
                    # one start=True per PSUM bank (clears the whole bank's
                    # has_written); later MMs overwrite untouched regions and
                    # accumulate written ones. I-MMs batched to share one LDW.
                    edge_ps = edge_pool.tile([128, G, C], FP32, name="edge_ps", tag="edge_ps")
                    for g in range(G):
                        for kch in range(CH):
                            nc.tensor.matmul(edge_ps[:, g, :],
                                             lhsT=attn_bf[:, kch, g, :],
                                             rhs=woe_sb[:, kch, :],
                                             start=(kch == 0 and g % 2 == 0),
                                             stop=False, skip_group_check=True)
                    for g in range(G):
                        nc.tensor.matmul(edge_ps[:, g, :], lhsT=ident[:],
                                         rhs=ynat_t[:, g, :], start=False,
                                         stop=(g % 2 == 1), skip_group_check=True)

                    # evacuate PSUM once, then ln4 off SBUF
                    r4sb = lp.tile([128, G, C], FP32, name="r4sb", tag="r4sb")
                    nc.vector.tensor_copy(out=r4sb.rearrange("p g c -> p (g c)"),
                                          in_=edge_ps.rearrange("p g c -> p (g c)"))
                    y2 = lp.tile([128, G, C], F32R, name="y2", tag="y2")
                    if apply_ln4sb:
                        layer_norm(y2, r4sb, "ln4", scale_t=ln4s, bias_t=ln4b, n_g=G)
                    else:
                        layer_norm(y2, r4sb, "ln4", n_g=G)

                    # y2 transposed via PE transpose-mode (bf16)
                    y2bf = lp.tile([128, G * C], BF16, name="y2bf", tag="y2bf")
                    nc.gpsimd.tensor_copy(out=y2bf, in_=y2.rearrange("p g c -> p (g c)"))
                    tps = edge_pool.tile([128, CH, G, 128], BF16, name="tps", tag="edge_ps")
                    for ch in range(CH):
                        for g in range(G):
                            nc.tensor.transpose(tps[:, ch, g, :],
                                                y2bf[:, g * C + ch * 128: g * C + (ch + 1) * 128],
                                                identb[:])
                    y2T = []
                    for ch in range(CH):
                        y2t_c = lp.tile([128, G * 128], BF16, name=f"y2T{ch}", tag=f"y2T{ch}")
                        if ch == 0:
                            nc.vector.tensor_copy(out=y2t_c, in_=tps[:, ch, :, :])
                        else:
                            nc.scalar.activation(out=y2t_c, in_=tps[:, ch, :, :],
                                                 func=AF.Identity, bias=0.0, scale=1.0)
                        y2T.append(y2t_c)

                    if _dbg:
                        for ch in range(CH):
                            ytf = lp.tile([128, G * 128], FP32, name=f"ytf{ch}", tag=f"ytf{ch}")
                            nc.vector.tensor_copy(out=ytf, in_=y2T[ch])
                            nc.sync.dma_start(out=dbg_yt_d[blk, ch], in_=ytf)

                    # mlp1: h^T = relu(W1^T-form + b1)
                    h_bf = lp.tile([128, MH, G * 128], BF16, name="h_bf", tag="h_bf")
                    for m in range(MH):
                        hp = h_pool.tile([128, G * 128], FP32, name="hp", tag="hp")
                        for kch in range(CH):
                            nc.tensor.matmul(hp[:], lhsT=w1y_sb[:, kch, m * 128:(m + 1) * 128],
                                             rhs=y2T[kch][:],
                                             start=(kch == 0), stop=(kch == CH - 1))
                        nc.scalar.activation(out=h_bf[:, m, :], in_=hp[:],
                                             func=AF.Relu, bias=b1y[:, m:m + 1], scale=1.0)

                    if _dbg:
                        for m in range(MH):
                            hf = lp.tile([128, G * 128], FP32, name=f"hf{m}", tag="hf", bufs=2)
                            nc.vector.tensor_copy(out=hf, in_=h_bf[:, m, :])
                            nc.sync.dma_start(out=dbg_h_d[blk, m], in_=hf)

                    # mlp2 natural + b2 + y2 residual in PSUM
                    r5_ps = r5_pool.tile([128, G, C], FP32, name="r5_ps", tag="r5_ps")
                    for g in range(G):
                        for m in range(MH):
                            nc.tensor.matmul(r5_ps[:, g, :],
                                             lhsT=h_bf[:, m, g * 128:(g + 1) * 128],
                                             rhs=w2y_sb[:, m, :],
                                             start=(m == 0 and g % 2 == 0),
                                             stop=False, skip_group_check=True)
                    for g in range(G):
                        nc.tensor.matmul(r5_ps[:, g, :], lhsT=ones1[:], rhs=b2y[:],
                                         start=False, stop=False, skip_group_check=True)
                    for g in range(G):
                        nc.tensor.matmul(r5_ps[:, g, :], lhsT=ident[:],
                                         rhs=y2[:, g, :], start=False,
                                         stop=(g % 2 == 1), skip_group_check=True)

                    if _dbg:
                        y2cp = lp.tile([128, G, C], FP32, name="y2cp", tag="y2cp")
                        nc.vector.tensor_copy(out=y2cp, in_=y2)
                        nc.sync.dma_start(out=dbg_y2_d[i0:i0 + G].rearrange("g j c -> j g c"), in_=y2cp)
                        r5cp = lp.tile([128, G, C], FP32, name="r5cp", tag="r5cp")
                        nc.vector.tensor_copy(out=r5cp, in_=r5_ps)
                        nc.sync.dma_start(out=dbg_r5_d[i0:i0 + G].rearrange("g j c -> j g c"), in_=r5cp)

                    # evacuate PSUM once, then ln6 off SBUF
                    r5sb = lp.tile([128, G, C], FP32, name="r5sb", tag="r5sb")
                    nc.scalar.activation(out=r5sb.rearrange("p g c -> p (g c)"),
                                         in_=r5_ps.rearrange("p g c -> p (g c)"),
                                         func=AF.Identity, bias=0.0, scale=1.0)
                    yout = lp3.tile([128, G, C], FP32, name="yout", tag="yout")
                    layer_norm(yout, r5sb, "ln6", n_g=G)
                    nc.sync.dma_start(out=yo_d[i0:i0 + G].rearrange("g j c -> j g c"),
                                      in_=yout)

            # ---------- x-stream tail ----------
            with (
                tc.tile_pool(name="tl", bufs=1) as tl,
                tc.tile_pool(name="tlps", bufs=1, space="PSUM") as tlps,
            ):
                node_ps = tlps.tile([128, C], FP32, name="node_ps")
                for kch in range(CH):
                    nc.tensor.matmul(node_ps[:], lhsT=agg[:, kch, :],
                                     rhs=won_sb[:, kch, :],
                                     start=(kch == 0), stop=False)
                nc.tensor.matmul(node_ps[:], lhsT=ones1[:], rhs=bon[:],
                                 start=False, stop=False)
                nc.tensor.matmul(node_ps[:], lhsT=ident[:], rhs=x1,
                                 start=False, stop=True)
                x2 = tl.tile([128, C], F32R, name="x2")
                layer_norm(x2, node_ps, "ln3", scale_t=ln3s, bias_t=ln3b)

                x2bf = tl.tile([128, C], BF16, name="x2bf")
                nc.vector.tensor_copy(out=x2bf, in_=x2)
                x2tp = tlps.tile([128, CH, 128], BF16, name="x2tp")
                x2T0 = tl.tile([128, 128], BF16, name="x2T0")
                x2T1 = tl.tile([128, 128], BF16, name="x2T1")
                x2Ts = [x2T0, x2T1]
                for ch in range(CH):
                    nc.tensor.transpose(x2tp[:, ch, :],
                                        x2bf[:, ch * 128:(ch + 1) * 128], identb[:])
                    nc.vector.tensor_copy(out=x2Ts[ch], in_=x2tp[:, ch, :])

                hx_bf = tl.tile([128, MH, 128], BF16, name="hx_bf")
                for m in range(MH):
                    hxp = tlps.tile([128, 128], FP32, name="hxp", tag="hxp", bufs=2)
                    for kch in range(CH):
                        nc.tensor.matmul(hxp[:], lhsT=w1x_sb[:, kch, m * 128:(m + 1) * 128],
                                         rhs=x2Ts[kch][:],
                                         start=(kch == 0), stop=(kch == CH - 1))
                    nc.scalar.activation(out=hx_bf[:, m, :], in_=hxp[:],
                                         func=AF.Relu, bias=b1x[:, m:m + 1], scale=1.0)

                mx_ps = tlps.tile([128, C], FP32, name="mx_ps")
                for m in range(MH):
                    nc.tensor.matmul(mx_ps[:], lhsT=hx_bf[:, m, :], rhs=w2x_sb[:, m, :],
                                     start=(m == 0), stop=False)
                nc.tensor.matmul(mx_ps[:], lhsT=ones1[:], rhs=b2x[:],
                                 start=False, stop=False)
                nc.tensor.matmul(mx_ps[:], lhsT=ident[:], rhs=x2,
                                 start=False, stop=True)
                xo_t = tl.tile([128, C], FP32, name="xo_t")
                layer_norm(xo_t, mx_ps, "ln5")
                nc.sync.dma_start(out=xo_d[:], in_=xo_t)

    nc.finalize()
    return nc


def kernel(x, y, Wq, bq, Wk, bk, Wv, bv, We, be, Woe, boe, Won, bon,
           mlp1_w1, mlp1_b1, mlp1_w2, mlp1_b2,
           mlp2_w1, mlp2_b1, mlp2_w2, mlp2_b2,
           ln1_s, ln1_b, ln3_s, ln3_b, ln4_s, ln4_b,
           ln5_s, ln5_b, ln6_s, ln6_b):
    x = np.asarray(x, np.float32)
    y = np.asarray(y, np.float32)

    def triv(s, b):
        return (np.allclose(np.asarray(s), 1.0, atol=0, rtol=0)
                and np.allclose(np.asarray(b), 0.0, atol=0, rtol=0))

    apply_ln4sb = not triv(ln4_s, ln4_b)
    import os
    key = (apply_ln4sb, bool(os.environ.get("KDEBUG")))
    if key not in _BUILD_CACHE:
        _BUILD_CACHE[key] = _build(apply_ln4sb)
    nc = _BUILD_CACHE[key]

    scale = 1.0 / np.sqrt(np.float32(DK))

    def wchunks(w, kdim=C):
        # [in, out] -> [in/128, 128, out]
        w = np.asarray(w, np.float32)
        return np.ascontiguousarray(w.reshape(kdim // 128, 128, -1))

    def bf(a):
        return np.ascontiguousarray(a).astype(BF16NP)

    wq = wchunks(np.asarray(Wq, np.float32) * scale)
    shared = {
        "we": wchunks(We).astype(np.float32),
        "woe": bf(wchunks(Woe)),
        "wq": bf(wq),
        "wk": bf(wchunks(Wk)),
        "wv": bf(wchunks(Wv)),
        "won": wchunks(Won).astype(np.float32),
        "w1y": bf(wchunks(mlp2_w1)),
        "w2y": bf(wchunks(mlp2_w2, kdim=HID)),
        "w1x": bf(wchunks(mlp1_w1)),
        "w2x": bf(wchunks(mlp1_w2, kdim=HID)),
        "bqs": np.ascontiguousarray((np.asarray(bq, np.float32) * scale).reshape(CH, 128).T),
        "bks": np.ascontiguousarray(np.asarray(bk, np.float32).reshape(CH, 128).T),
        "bvs": np.ascontiguousarray(np.asarray(bv, np.float32).reshape(CH, 128).T),
        "be5": np.ascontiguousarray((np.asarray(be, np.float32) + 0.5).reshape(CH, 128).T),
        "b1y": np.ascontiguousarray(np.asarray(mlp2_b1, np.float32).reshape(MH, 128).T),
        "b1x": np.ascontiguousarray(np.asarray(mlp1_b1, np.float32).reshape(MH, 128).T),
        "boe": np.asarray(boe, np.float32).reshape(1, C),
        "b2y": np.asarray(mlp2_b2, np.float32).reshape(1, C),
        "bon": np.asarray(bon, np.float32).reshape(1, C),
        "b2x": np.asarray(mlp1_b2, np.float32).reshape(1, C),
        "ones1": np.ones((1, 128), np.float32),
        "ident": np.eye(128, dtype=np.float32),
        "identb": np.eye(128, dtype=np.float32).astype(BF16NP),
        "ln1s": np.asarray(ln1_s, np.float32).reshape(1, C),
        "ln1b": np.asarray(ln1_b, np.float32).reshape(1, C),
        "ln3s": np.asarray(ln3_s, np.float32).reshape(1, C),
        "ln3b": np.asarray(ln3_b, np.float32).reshape(1, C),
        "ln4s": np.asarray(ln4_s, np.float32).reshape(1, C),
        "ln4b": np.asarray(ln4_b, np.float32).reshape(1, C),
    }

    boe_f = np.asarray(boe, np.float32).reshape(1, 1, C)

    # host transpose of y: ytr[blk, ch, p, g, j] = y[b, blk*G+g, j, ch*128+p]
    # y[b]: (i, j, c) -> (i, c, j) -> (blk, g, ch, p, j) -> (blk, ch, p, g, j)
    ytr_all = (y.transpose(0, 1, 3, 2)
                .reshape(B, NBLK, G, CH, 128, 128)
                .transpose(0, 1, 3, 4, 2, 5))

    in_maps = []
    for b in range(B):
        m = dict(shared)
        m["x"] = np.ascontiguousarray(x[b])
        m["ynat"] = np.ascontiguousarray(y[b] + boe_f)
        m["ytr"] = np.ascontiguousarray(ytr_all[b])
        in_maps.append(m)

    r = run_bass_kernel_spmd(nc, in_maps, core_ids=list(range(B)))

    x_out = np.stack([r.results[b]["xo"] for b in range(B)])
    y_out = np.stack([r.results[b]["yo"] for b in range(B)])

    if not triv(ln5_s, ln5_b):
        x_out = x_out * np.asarray(ln5_s, np.float32) + np.asarray(ln5_b, np.float32)
    if not triv(ln6_s, ln6_b):
        y_out = y_out * np.asarray(ln6_s, np.float32) + np.asarray(ln6_b, np.float32)

    return (x_out.astype(np.float32), y_out.astype(np.float32))


# revision 27
# speedup vs baseline: 1.0026x; 1.0026x over previous
"""Trainium2 Bass kernel for nn_Encoder_Block (graph-transformer encoder block).

Sharding: data-parallel over batch b — core c handles batch c (B=8, 8 cores).

Per-core dataflow (b fixed; N=128 nodes, C=256, H*DK=C, HID=1024):
  phase 0 : x1 = LN1(x); qT/kT/vT = (x1 @ Wqkv + b)^T via PE (x1 transposed
            through a DRAM round-trip + DMA-xbar transpose).
  main loop over i-blocks of G=4 rows of the edge tensor:
    eT   = We^T-form matmul on host-pre-transposed y (f32r, full fp32 bits)
    u    = Square(e + be + 0.5)  (ACT, per-partition bias)     [bf16]
    uq   = (u - 0.25) * q_i      (DVE fused tensor_scalar)
    attn = uq * kT               (DVE tensor_tensor, bcast AP over g)
    s    = Exp(attn)             (ACT)
    sumexp / wsum = fused tensor_scalar / scalar_tensor_tensor accum_out
    agg column = wsum * recip(sumexp)
    edge_out natural [j,c] via attn-as-lhsT matmuls + boe ones-row matmul
            + y residual via identity matmul (f32r) accumulated in PSUM
    ln4  = bn_stats(PSUM) + ln/exp rstd + fused tensor_scalar apply -> y2
    y2T  via DRAM round-trip + xbar transpose (contiguous [128,512] dests)
    mlp  = relu(W1^T-form) (ACT bias) -> h^T ; out natural via h-as-lhsT
            + b2 ones-row + y2 identity-residual in PSUM
    ln6  -> y_out -> DMA out
  tail    : node_out from agg, ln3, mlp_x, ln5 -> x_out.

LN scale/bias: setup_inputs uses s=1, b=0 for all LNs. ln1/ln3 are applied
generally on device (one-time broadcast tiles). ln4 gets a device fallback
path if nontrivial; ln5/ln6 are host-fixable (final ops).
"""
import sys
import numpy as np

sys.path.insert(0, "/opt/trn_rl_repo")

import ml_dtypes  # noqa: E402

import concourse.bacc as bacc  # noqa: E402
import concourse.bass as bass  # noqa: E402
import concourse.tile as tile  # noqa: E402
from concourse import mybir  # noqa: E402
from concourse.bass_utils import run_bass_kernel_spmd  # noqa: E402

# Steer every ACT function this kernel uses into one table set
# (natural_log_exp_and_others) so the whole kernel runs with a single
# ACT_TABLE_LOAD instead of ping-ponging between sets on every Ln/Exp.
# Only the python-side chooser dict is edited; set ids (insertion order)
# stay aligned with the act_info.json walrus reads, so the loads remain
# valid — the chooser just loses the option of picking a set that lacks
# one of our functions.
import concourse.hw_specs as _hw_specs  # noqa: E402

_ORIG_GAT = _hw_specs.get_activation_tables
_MY_SET = "natural_log_exp_and_others"


def _patched_gat(arch):
    tables = _ORIG_GAT(arch)
    mine = tables.get(_MY_SET)
    if mine:
        my_funcs = {
            mybir.ActivationFunctionType.Square,
            mybir.ActivationFunctionType.Exp,
            mybir.ActivationFunctionType.Ln,
            mybir.ActivationFunctionType.Relu,
            mybir.ActivationFunctionType.Identity,
        } & mine
        tables = {
            name: (funcs if name == _MY_SET else funcs - my_funcs)
            for name, funcs in tables.items()
        }
    return tables


bacc.get_activation_tables = _patched_gat

FP32 = mybir.dt.float32
F32R = mybir.dt.float32r
BF16 = mybir.dt.bfloat16
AF = mybir.ActivationFunctionType
OP = mybir.AluOpType
BF16NP = ml_dtypes.bfloat16

B, N, C, HID = 8, 128, 256, 1024
DK = 32
G = 4                      # i-rows per block
NBLK = N // G              # 32
CH = C // 128              # 2 chunks of the feature dim
MH = HID // 128            # 8 chunks of the mlp hidden dim
EPS = 1e-5

_BUILD_CACHE = {}


def _bcast_g(ap_2d, g):
    """[p, f] AP -> [p, g, f] with stride-0 middle dim."""
    return bass.AP(tensor=ap_2d.tensor, offset=ap_2d.offset,
                   ap=[ap_2d.ap[0], [0, g], ap_2d.ap[1]])


def _build(apply_ln4sb: bool):
    nc = bacc.Bacc()

    dp = nc.declare_dram_parameter
    x_d = dp("x", [N, C], FP32, isOutput=False)
    ynat_d = dp("ynat", [N, N, C], F32R, isOutput=False)
    ytr_d = dp("ytr", [NBLK, CH, 128, G, 128], F32R, isOutput=False)
    we_d = dp("we", [CH, 128, C], F32R, isOutput=False)
    woe_d = dp("woe", [CH, 128, C], BF16, isOutput=False)
    wq_d = dp("wq", [CH, 128, C], BF16, isOutput=False)
    wk_d = dp("wk", [CH, 128, C], BF16, isOutput=False)
    wv_d = dp("wv", [CH, 128, C], BF16, isOutput=False)
    won_d = dp("won", [CH, 128, C], F32R, isOutput=False)
    w1y_d = dp("w1y", [CH, 128, HID], BF16, isOutput=False)
    w2y_d = dp("w2y", [MH, 128, C], BF16, isOutput=False)
    w1x_d = dp("w1x", [CH, 128, HID], BF16, isOutput=False)
    w2x_d = dp("w2x", [MH, 128, C], BF16, isOutput=False)
    bqs_d = dp("bqs", [128, CH], FP32, isOutput=False)
    bks_d = dp("bks", [128, CH], FP32, isOutput=False)
    bvs_d = dp("bvs", [128, CH], FP32, isOutput=False)
    be5_d = dp("be5", [128, CH], FP32, isOutput=False)
    b1y_d = dp("b1y", [128, MH], FP32, isOutput=False)
    b1x_d = dp("b1x", [128, MH], FP32, isOutput=False)
    b2y_d = dp("b2y", [1, C], F32R, isOutput=False)
    bon_d = dp("bon", [1, C], F32R, isOutput=False)
    b2x_d = dp("b2x", [1, C], F32R, isOutput=False)
    ones_d = dp("ones1", [1, 128], F32R, isOutput=False)
    ident_d = dp("ident", [128, 128], F32R, isOutput=False)
    identb_d = dp("identb", [128, 128], BF16, isOutput=False)
    ln1s_d = dp("ln1s", [1, C], FP32, isOutput=False)
    ln1b_d = dp("ln1b", [1, C], FP32, isOutput=False)
    ln3s_d = dp("ln3s", [1, C], FP32, isOutput=False)
    ln3b_d = dp("ln3b", [1, C], FP32, isOutput=False)
    ln4s_d = dp("ln4s", [1, C], FP32, isOutput=False)
    ln4b_d = dp("ln4b", [1, C], FP32, isOutput=False)

    xo_d = dp("xo", [N, C], FP32, isOutput=True)
    yo_d = dp("yo", [N, N, C], FP32, isOutput=True)
    import os
    _dbg = bool(os.environ.get("KDEBUG"))
    if _dbg:
        dbg_y2_d = dp("dbg_y2", [N, N, C], FP32, isOutput=True)
        dbg_r5_d = dp("dbg_r5", [N, N, C], FP32, isOutput=True)
        dbg_yt_d = dp("dbg_yt", [NBLK, CH, 128, G * 128], FP32, isOutput=True)
        dbg_h_d = dp("dbg_h", [NBLK, MH, 128, G * 128], FP32, isOutput=True)

    with tile.TileContext(nc) as tc:
        with (
            tc.tile_pool(name="persist", bufs=1) as pp,
        ):
            # ---------- persistent loads ----------
            def pload(name, dram, shape, dt):
                t = pp.tile(shape, dt, name=name, tag=name)
                nc.sync.dma_start(out=t, in_=dram[:].rearrange("k p c -> p k c"))
                return t

            wq_sb = pload("wq_sb", wq_d, [128, CH, C], BF16)
            wk_sb = pload("wk_sb", wk_d, [128, CH, C], BF16)
            wv_sb = pload("wv_sb", wv_d, [128, CH, C], BF16)
            we_sb = pload("we_sb", we_d, [128, CH, C], F32R)
            woe_sb = pload("woe_sb", woe_d, [128, CH, C], BF16)
            w1y_sb = pload("w1y_sb", w1y_d, [128, CH, HID], BF16)
            w2y_sb = pload("w2y_sb", w2y_d, [128, MH, C], BF16)
            won_sb = pload("won_sb", won_d, [128, CH, C], F32R)
            w1x_sb = pload("w1x_sb", w1x_d, [128, CH, HID], BF16)
            w2x_sb = pload("w2x_sb", w2x_d, [128, MH, C], BF16)

            def sload(name, dram, shape, dt):
                t = pp.tile(shape, dt, name=name, tag=name)
                nc.sync.dma_start(out=t, in_=dram[:])
                return t

            bqs = sload("bqs_sb", bqs_d, [128, CH], FP32)
            bks = sload("bks_sb", bks_d, [128, CH], FP32)
            bvs = sload("bvs_sb", bvs_d, [128, CH], FP32)
            be5 = sload("be5_sb", be5_d, [128, CH], FP32)
            b1y = sload("b1y_sb", b1y_d, [128, MH], FP32)
            b1x = sload("b1x_sb", b1x_d, [128, MH], FP32)
            b2y = sload("b2y_sb", b2y_d, [1, C], F32R)
            bon = sload("bon_sb", bon_d, [1, C], F32R)
            b2x = sload("b2x_sb", b2x_d, [1, C], F32R)
            ones1 = sload("ones1_sb", ones_d, [1, 128], F32R)
            ident = sload("ident_sb", ident_d, [128, 128], F32R)
            identb = sload("identb_sb", identb_d, [128, 128], BF16)

            def bcload(name, dram):
                t = pp.tile([128, C], FP32, name=name, tag=name)
                src = dram[:]
                bc = bass.AP(tensor=src.tensor, offset=src.offset,
                             ap=[[0, 128], src.ap[1]])
                nc.gpsimd.dma_start(out=t, in_=bc)
                return t

            ln1s = bcload("ln1s_sb", ln1s_d)
            ln1b = bcload("ln1b_sb", ln1b_d)
            ln3s = bcload("ln3s_sb", ln3s_d)
            ln3b = bcload("ln3b_sb", ln3b_d)
            if apply_ln4sb:
                ln4s = bcload("ln4s_sb", ln4s_d)
                ln4b = bcload("ln4b_sb", ln4b_d)

            epsb = pp.tile([128, 1], FP32, name="epsb", tag="epsb")
            nc.vector.memset(epsb, EPS)

            # persistent x-stream tiles
            x1 = pp.tile([128, C], F32R, name="x1t", tag="x1t")
            qs = pp.tile([128, CH, 128], FP32, name="qs", tag="qs")
            kT = pp.tile([128, CH, 128], BF16, name="kT", tag="kT")
            vT = pp.tile([128, CH, 128], BF16, name="vT", tag="vT")
            agg = pp.tile([128, CH, 128], F32R, name="agg", tag="agg")
            x1T0 = pp.tile([128, 128], BF16, name="x1T0", tag="x1T0")
            x1T1 = pp.tile([128, 128], BF16, name="x1T1", tag="x1T1")
            x1Ts = [x1T0, x1T1]

            def layer_norm(dst, src, tagp, scale_t=None, bias_t=None, n_g=1,
                           g_len=C):
                """LN over last axis for [128, n_g, g_len] src (may be PSUM).
                dst fp32 SBUF same shape. Returns nothing."""
                mv = pp.tile([128, n_g, 2], FP32, name=f"{tagp}_mv", tag=f"{tagp}_mv")
                for g in range(n_g):
                    st = pp.tile([128, 6], FP32, name=f"{tagp}_st{g}", tag=f"{tagp}_st", bufs=2)
                    src_g = src[:, g, :] if n_g > 1 else src
                    nc.vector.bn_stats(out=st, in_=src_g)
                    nc.vector.bn_aggr(out=mv[:, g, :], in_=st)
                lnv = pp.tile([128, n_g], FP32, name=f"{tagp}_lnv", tag=f"{tagp}_lnv")
                nc.scalar.activation(out=lnv, in_=mv[:, :, 1], func=AF.Ln,
                                     bias=epsb[:], scale=1.0)
                rstd = pp.tile([128, n_g], FP32, name=f"{tagp}_rstd", tag=f"{tagp}_rstd")
                nc.scalar.activation(out=rstd, in_=lnv, func=AF.Exp,
                                     bias=0.0, scale=-0.5)
                for g in range(n_g):
                    src_g = src[:, g, :] if n_g > 1 else src
                    dst_g = dst[:, g, :] if n_g > 1 else dst
                    nc.vector.tensor_scalar(out=dst_g, in0=src_g,
                                            scalar1=mv[:, g, 0:1],
                                            scalar2=rstd[:, g:g + 1],
                                            op0=OP.subtract, op1=OP.mult)
                if scale_t is not None:
                    for g in range(n_g):
                        dst_g = dst[:, g, :] if n_g > 1 else dst
                        nc.vector.tensor_tensor(out=dst_g, in0=dst_g, in1=scale_t, op=OP.mult)
                        nc.vector.tensor_tensor(out=dst_g, in0=dst_g, in1=bias_t, op=OP.add)

            # ---------- phase 0: x1, qT/kT/vT ----------
            with (
                tc.tile_pool(name="p0", bufs=1) as p0,
                tc.tile_pool(name="p0ps", bufs=1, space="PSUM") as p0ps,
            ):
                xt = p0.tile([128, C], FP32, name="xt")
                nc.sync.dma_start(out=xt, in_=x_d[:])
                layer_norm(x1, xt, "ln1", scale_t=ln1s, bias_t=ln1b)

                x1bf = p0.tile([128, C], BF16, name="x1bf")
                nc.vector.tensor_copy(out=x1bf, in_=x1)
                x1tp = p0ps.tile([128, CH, 128], BF16, name="x1tp")
                for ch in range(CH):
                    nc.tensor.transpose(x1tp[:, ch, :],
                                        x1bf[:, ch * 128:(ch + 1) * 128], identb[:])
                    nc.vector.tensor_copy(out=x1Ts[ch], in_=x1tp[:, ch, :])

                for (w_sb, b_sb, outt, odt) in ((wq_sb, bqs, qs, FP32),
                                                (wk_sb, bks, kT, BF16),
                                                (wv_sb, bvs, vT, BF16)):
                    for mch in range(CH):
                        qps = p0ps.tile([128, 128], FP32, name="qps", tag="qps")
                        for kch in range(CH):
                            nc.tensor.matmul(qps[:], lhsT=w_sb[:, kch, mch * 128:(mch + 1) * 128],
                                             rhs=x1Ts[kch][:],
                                             start=(kch == 0), stop=(kch == CH - 1))
                        nc.scalar.activation(out=outt[:, mch, :], in_=qps[:],
                                             func=AF.Identity, bias=b_sb[:, mch:mch + 1],
                                             scale=1.0)

            # ---------- main loop over edge-row blocks ----------
            with (
                tc.tile_pool(name="lp", bufs=(2 if _dbg else 3)) as lp,
                tc.tile_pool(name="lp3", bufs=(2 if _dbg else 3)) as lp3,
                tc.tile_pool(name="eps_pool", bufs=2, space="PSUM") as eps_pool,
                tc.tile_pool(name="edge_pool", bufs=1, space="PSUM") as edge_pool,
                tc.tile_pool(name="h_pool", bufs=2, space="PSUM") as h_pool,
                tc.tile_pool(name="r5_pool", bufs=1, space="PSUM") as r5_pool,
            ):
                for blk in range(NBLK):
                    i0 = blk * G
                    # loads
                    ytr_t = lp.tile([128, CH, G, 128], F32R, name="ytr_t", tag="ytr_t")
                    nc.sync.dma_start(out=ytr_t, in_=ytr_d[blk].rearrange("k p g j -> p k g j"))
                    ynat_t = lp.tile([128, G, C], F32R, name="ynat_t", tag="ynat_t")
                    nc.sync.dma_start(out=ynat_t, in_=ynat_d[i0:i0 + G].rearrange("g j c -> j g c"))

                    # e projection (transposed, f32r)
                    e_ps = []
                    for mch in range(CH):
                        ep = eps_pool.tile([128, G * 128], FP32, name=f"e_ps{mch}", tag="e_ps")
                        for kch in range(CH):
                            nc.tensor.matmul(ep[:], lhsT=we_sb[:, kch, mch * 128:(mch + 1) * 128],
                                             rhs=ytr_t[:, kch, :, :],
                                             start=(kch == 0), stop=(kch == CH - 1))
                        e_ps.append(ep)

                    # u = (e + be + 0.5)^2  [bf16]
                    u_bf = lp.tile([128, CH, G * 128], BF16, name="u_bf", tag="u_bf", bufs=4)
                    for mch in range(CH):
                        nc.scalar.activation(out=u_bf[:, mch, :], in_=e_ps[mch][:],
                                             func=AF.Square, bias=be5[:, mch:mch + 1],
                                             scale=1.0)

                    # uq = (u - 0.25) * q_i ; attn = uq * kT
                    uq_bf = lp.tile([128, CH, G, 128], BF16, name="uq_bf", tag="uq_bf")
                    for mch in range(CH):
                        for g in range(G):
                            nc.vector.tensor_scalar(
                                out=uq_bf[:, mch, g, :],
                                in0=u_bf[:, mch, g * 128:(g + 1) * 128],
                                scalar1=0.25, scalar2=qs[:, mch, i0 + g:i0 + g + 1],
                                op0=OP.subtract, op1=OP.mult)
                    attn_bf = lp.tile([128, CH, G, 128], BF16, name="attn_bf", tag="attn_bf", bufs=4)
                    for mch in range(CH):
                        nc.vector.tensor_tensor(out=attn_bf[:, mch, :, :],
                                                in0=uq_bf[:, mch, :, :],
                                                in1=_bcast_g(kT[:, mch, :], G), op=OP.mult)

                    # softmax pieces
                    s_bf = lp.tile([128, CH, G, 128], BF16, name="s_bf", tag="s_bf", bufs=4)
                    nc.scalar.activation(out=s_bf.rearrange("p m g j -> p (m g j)"),
                                         in_=attn_bf.rearrange("p m g j -> p (m g j)"),
                                         func=AF.Exp, bias=0.0, scale=1.0)
                    se = lp.tile([128, CH, G], FP32, name="se", tag="se")
                    ws = lp.tile([128, CH, G], FP32, name="ws", tag="ws")
                    scr = lp.tile([128, 128], BF16, name="scr", tag="scr")
                    scr2 = lp.tile([128, 128], BF16, name="scr2", tag="scr2")
                    for mch in range(CH):
                        for g in range(G):
                            nc.vector.tensor_scalar(out=scr, in0=s_bf[:, mch, g, :],
                                                    scalar1=1.0, scalar2=0.0,
                                                    op0=OP.mult, op1=OP.add,
                                                    accum_out=se[:, mch, g:g + 1])
                            nc.vector.scalar_tensor_tensor(out=scr2, in0=s_bf[:, mch, g, :],
                                                           scalar=1.0, in1=vT[:, mch, :],
                                                           op0=OP.mult, op1=OP.mult,
                                                           accum_out=ws[:, mch, g:g + 1])
                    rse = lp.tile([128, CH, G], FP32, name="rse", tag="rse")
                    nc.vector.reciprocal(out=rse, in_=se)
                    for mch in range(CH):
                        nc.vector.tensor_tensor(out=agg[:, mch, i0:i0 + G],
                                                in0=ws[:, mch, :], in1=rse[:, mch, :],
                                                op=OP.mult)

                    # edge_out natural + boe + y residual, accumulate in PSUM
                    # one start=True per PSUM bank (clears the whole bank's
                    # has_written); later MMs overwrite untouched regions and
                    # accumulate written ones. I-MMs batched to share one LDW.
                    edge_ps = edge_pool.tile([128, G, C], FP32, name="edge_ps", tag="edge_ps")
                    for g in range(G):
                        for kch in range(CH):
                            nc.tensor.matmul(edge_ps[:, g, :],
                                             lhsT=attn_bf[:, kch, g, :],
                                             rhs=woe_sb[:, kch, :],
                                             start=(kch == 0 and g % 2 == 0),
                                             stop=False, skip_group_check=True)
                    for g in range(G):
                        nc.tensor.matmul(edge_ps[:, g, :], lhsT=ident[:],
                                         rhs=ynat_t[:, g, :], start=False,
                                         stop=(g % 2 == 1), skip_group_check=True)

                    # evacuate PSUM once, then ln4 off SBUF
                    r4sb = lp.tile([128, G, C], FP32, name="r4sb", tag="r4sb")
                    nc.vector.tensor_copy(out=r4sb.rearrange("p g c -> p (g c)"),
                                          in_=edge_ps.rearrange("p g c -> p (g c)"))
                    y2 = lp.tile([128, G, C], F32R, name="y2", tag="y2")
                    if apply_ln4sb:
                        layer_norm(y2, r4sb, "ln4", scale_t=ln4s, bias_t=ln4b, n_g=G)
                    else:
                        layer_norm(y2, r4sb, "ln4", n_g=G)

                    # y2 transposed via PE transpose-mode (bf16)
                    y2bf = lp.tile([128, G * C], BF16, name="y2bf", tag="y2bf")
                    nc.gpsimd.tensor_copy(out=y2bf, in_=y2.rearrange("p g c -> p (g c)"))
                    tps = edge_pool.tile([128, CH, G, 128], BF16, name="tps", tag="edge_ps")
                    for ch in range(CH):
                        for g in range(G):
                            nc.tensor.transpose(tps[:, ch, g, :],
                                                y2bf[:, g * C + ch * 128: g * C + (ch + 1) * 128],
                                                identb[:])
                    y2T = []
                    for ch in range(CH):
                        y2t_c = lp.tile([128, G * 128], BF16, name=f"y2T{ch}", tag=f"y2T{ch}")
                        if ch == 0:
                            nc.vector.tensor_copy(out=y2t_c, in_=tps[:, ch, :, :])
                        else:
                            nc.scalar.activation(out=y2t_c, in_=tps[:, ch, :, :],
                                                 func=AF.Identity, bias=0.0, scale=1.0)
                        y2T.append(y2t_c)

                    if _dbg:
                        for ch in range(CH):
                            ytf = lp.tile([128, G * 128], FP32, name=f"ytf{ch}", tag=f"ytf{ch}")
                            nc.vector.tensor_copy(out=ytf, in_=y2T[ch])
                            nc.sync.dma_start(out=dbg_yt_d[blk, ch], in_=ytf)

                    # mlp1: h^T = relu(W1^T-form + b1)
                    h_bf = lp.tile([128, MH, G * 128], BF16, name="h_bf", tag="h_bf")
                    for m in range(MH):
                        hp = h_pool.tile([128, G * 128], FP32, name="hp", tag="hp")
                        for kch in range(CH):
                            nc.tensor.matmul(hp[:], lhsT=w1y_sb[:, kch, m * 128:(m + 1) * 128],
                                             rhs=y2T[kch][:],
                                             start=(kch == 0), stop=(kch == CH - 1))
                        nc.scalar.activation(out=h_bf[:, m, :], in_=hp[:],
                                             func=AF.Relu, bias=b1y[:, m:m + 1], scale=1.0)

                    if _dbg:
                        for m in range(MH):
                            hf = lp.tile([128, G * 128], FP32, name=f"hf{m}", tag="hf", bufs=2)
                            nc.vector.tensor_copy(out=hf, in_=h_bf[:, m, :])
                            nc.sync.dma_start(out=dbg_h_d[blk, m], in_=hf)

                    # mlp2 natural + b2 + y2 residual in PSUM
                    r5_ps = r5_pool.tile([128, G, C], FP32, name="r5_ps", tag="r5_ps")
                    for g in range(G):
                        for m in range(MH):
                            nc.tensor.matmul(r5_ps[:, g, :],
                                             lhsT=h_bf[:, m, g * 128:(g + 1) * 128],
                                             rhs=w2y_sb[:, m, :],
                                             start=(m == 0 and g % 2 == 0),
                                             stop=False, skip_group_check=True)
                    for g in range(G):
                        nc.tensor.matmul(r5_ps[:, g, :], lhsT=ones1[:], rhs=b2y[:],
                                         start=False, stop=False, skip_group_check=True)
                    for g in range(G):
                        nc.tensor.matmul(r5_ps[:, g, :], lhsT=ident[:],
                                         rhs=y2[:, g, :], start=False,
                                         stop=(g % 2 == 1), skip_group_check=True)

                    if _dbg:
                        y2cp = lp.tile([128, G, C], FP32, name="y2cp", tag="y2cp")
                        nc.vector.tensor_copy(out=y2cp, in_=y2)
                        nc.sync.dma_start(out=dbg_y2_d[i0:i0 + G].rearrange("g j c -> j g c"), in_=y2cp)
                        r5cp = lp.tile([128, G, C], FP32, name="r5cp", tag="r5cp")
                        nc.vector.tensor_copy(out=r5cp, in_=r5_ps)
                        nc.sync.dma_start(out=dbg_r5_d[i0:i0 + G].rearrange("g j c -> j g c"), in_=r5cp)

                    # evacuate PSUM once, then ln6 off SBUF
                    r5sb = lp.tile([128, G, C], FP32, name="r5sb", tag="r5sb")
                    nc.scalar.activation(out=r5sb.rearrange("p g c -> p (g c)"),
                                         in_=r5_ps.rearrange("p g c -> p (g c)"),
                                         func=AF.Identity, bias=0.0, scale=1.0)
                    yout = lp3.tile([128, G, C], FP32, name="yout", tag="yout")
                    layer_norm(yout, r5sb, "ln6", n_g=G)
                    nc.sync.dma_start(out=yo_d[i0:i0 + G].rearrange("g j c -> j g c"),
                                      in_=yout)

            # ---------- x-stream tail ----------
            with (
                tc.tile_pool(name="tl", bufs=1) as tl,
                tc.tile_pool(name="tlps", bufs=1, space="PSUM") as tlps,
            ):
                node_ps = tlps.tile([128, C], FP32, name="node_ps")
                for kch in range(CH):
                    nc.tensor.matmul(node_ps[:], lhsT=agg[:, kch, :],
                                     rhs=won_sb[:, kch, :],
                                     start=(kch == 0), stop=False)
                nc.tensor.matmul(node_ps[:], lhsT=ones1[:], rhs=bon[:],
                                 start=False, stop=False)
                nc.tensor.matmul(node_ps[:], lhsT=ident[:], rhs=x1,
                                 start=False, stop=True)
                x2 = tl.tile([128, C], F32R, name="x2")
                layer_norm(x2, node_ps, "ln3", scale_t=ln3s, bias_t=ln3b)

                x2bf = tl.tile([128, C], BF16, name="x2bf")
                nc.vector.tensor_copy(out=x2bf, in_=x2)
                x2tp = tlps.tile([128, CH, 128], BF16, name="x2tp")
                x2T0 = tl.tile([128, 128], BF16, name="x2T0")
                x2T1 = tl.tile([128, 128], BF16, name="x2T1")
                x2Ts = [x2T0, x2T1]
                for ch in range(CH):
                    nc.tensor.transpose(x2tp[:, ch, :],
                                        x2bf[:, ch * 128:(ch + 1) * 128], identb[:])
                    nc.vector.tensor_copy(out=x2Ts[ch], in_=x2tp[:, ch, :])

                hx_bf = tl.tile([128, MH, 128], BF16, name="hx_bf")
                for m in range(MH):
                    hxp = tlps.tile([128, 128], FP32, name="hxp", tag="hxp", bufs=2)
                    for kch in range(CH):
                        nc.tensor.matmul(hxp[:], lhsT=w1x_sb[:, kch, m * 128:(m + 1) * 128],
                                         rhs=x2Ts[kch][:],
                                         start=(kch == 0), stop=(kch == CH - 1))
                    nc.scalar.activation(out=hx_bf[:, m, :], in_=hxp[:],
                                         func=AF.Relu, bias=b1x[:, m:m + 1], scale=1.0)

                mx_ps = tlps.tile([128, C], FP32, name="mx_ps")
                for m in range(MH):
                    nc.tensor.matmul(mx_ps[:], lhsT=hx_bf[:, m, :], rhs=w2x_sb[:, m, :],
                                     start=(m == 0), stop=False)
                nc.tensor.matmul(mx_ps[:], lhsT=ones1[:], rhs=b2x[:],
                                 start=False, stop=False)
                nc.tensor.matmul(mx_ps[:], lhsT=ident[:], rhs=x2,
                                 start=False, stop=True)
                xo_t = tl.tile([128, C], FP32, name="xo_t")
                layer_norm(xo_t, mx_ps, "ln5")
                nc.sync.dma_start(out=xo_d[:], in_=xo_t)

    nc.finalize()
    return nc


def kernel(x, y, Wq, bq, Wk, bk, Wv, bv, We, be, Woe, boe, Won, bon,
           mlp1_w1, mlp1_b1, mlp1_w2, mlp1_b2,
           mlp2_w1, mlp2_b1, mlp2_w2, mlp2_b2,
           ln1_s, ln1_b, ln3_s, ln3_b, ln4_s, ln4_b,
           ln5_s, ln5_b, ln6_s, ln6_b):
    x = np.asarray(x, np.float32)
    y = np.asarray(y, np.float32)

    def triv(s, b):
        return (np.allclose(np.asarray(s), 1.0, atol=0, rtol=0)
                and np.allclose(np.asarray(b), 0.0, atol=0, rtol=0))

    apply_ln4sb = not triv(ln4_s, ln4_b)
    import os
    key = (apply_ln4sb, bool(os.environ.get("KDEBUG")))
    if key not in _BUILD_CACHE:
        _BUILD_CACHE[key] = _build(apply_ln4sb)
    nc = _BUILD_CACHE[key]

    scale = 1.0 / np.sqrt(np.float32(DK))

    def wchunks(w, kdim=C):
        # [in, out] -> [in/128, 128, out]
        w = np.asarray(w, np.float32)
        return np.ascontiguousarray(w.reshape(kdim // 128, 128, -1))

    def bf(a):
        return np.ascontiguousarray(a).astype(BF16NP)

    wq = wchunks(np.asarray(Wq, np.float32) * scale)
    shared = {
        "we": wchunks(We).astype(np.float32),
        "woe": bf(wchunks(Woe)),
        "wq": bf(wq),
        "wk": bf(wchunks(Wk)),
        "wv": bf(wchunks(Wv)),
        "won": wchunks(Won).astype(np.float32),
        "w1y": bf(wchunks(mlp2_w1)),
        "w2y": bf(wchunks(mlp2_w2, kdim=HID)),
        "w1x": bf(wchunks(mlp1_w1)),
        "w2x": bf(wchunks(mlp1_w2, kdim=HID)),
        "bqs": np.ascontiguousarray((np.asarray(bq, np.float32) * scale).reshape(CH, 128).T),
        "bks": np.ascontiguousarray(np.asarray(bk, np.float32).reshape(CH, 128).T),
        "bvs": np.ascontiguousarray(np.asarray(bv, np.float32).reshape(CH, 128).T),
        "be5": np.ascontiguousarray((np.asarray(be, np.float32) + 0.5).reshape(CH, 128).T),
        "b1y": np.ascontiguousarray(np.asarray(mlp2_b1, np.float32).reshape(MH, 128).T),
        "b1x": np.ascontiguousarray(np.asarray(mlp1_b1, np.float32).reshape(MH, 128).T),
        "b2y": np.asarray(mlp2_b2, np.float32).reshape(1, C),
        "bon": np.asarray(bon, np.float32).reshape(1, C),
        "b2x": np.asarray(mlp1_b2, np.float32).reshape(1, C),
        "ones1": np.ones((1, 128), np.float32),
        "ident": np.eye(128, dtype=np.float32),
        "identb": np.eye(128, dtype=np.float32).astype(BF16NP),
        "ln1s": np.asarray(ln1_s, np.float32).reshape(1, C),
        "ln1b": np.asarray(ln1_b, np.float32).reshape(1, C),
        "ln3s": np.asarray(ln3_s, np.float32).reshape(1, C),
        "ln3b": np.asarray(ln3_b, np.float32).reshape(1, C),
        "ln4s": np.asarray(ln4_s, np.float32).reshape(1, C),
        "ln4b": np.asarray(ln4_b, np.float32).reshape(1, C),
    }

    boe_f = np.asarray(boe, np.float32).reshape(1, 1, C)

    # host transpose of y: ytr[blk, ch, p, g, j] = y[b, blk*G+g, j, ch*128+p]
    # y[b]: (i, j, c) -> (i, c, j) -> (blk, g, ch, p, j) -> (blk, ch, p, g, j)
    ytr_all = (y.transpose(0, 1, 3, 2)
                .reshape(B, NBLK, G, CH, 128, 128)
                .transpose(0, 1, 3, 4, 2, 5))

    in_maps = []
    for b in range(B):
        m = dict(shared)
        m["x"] = np.ascontiguousarray(x[b])
        m["ynat"] = np.ascontiguousarray(y[b] + boe_f)
        m["ytr"] = np.ascontiguousarray(ytr_all[b])
        in_maps.append(m)

    r = run_bass_kernel_spmd(nc, in_maps, core_ids=list(range(B)))

    x_out = np.stack([r.results[b]["xo"] for b in range(B)])
    y_out = np.stack([r.results[b]["yo"] for b in range(B)])

    if not triv(ln5_s, ln5_b):
        x_out = x_out * np.asarray(ln5_s, np.float32) + np.asarray(ln5_b, np.float32)
    if not triv(ln6_s, ln6_b):
        y_out = y_out * np.asarray(ln6_s, np.float32) + np.asarray(ln6_b, np.float32)

    return (x_out.astype(np.float32), y_out.astype(np.float32))
